# revision 40
# baseline (speedup 1.0000x reference)
"""Longformer decoder (4 layers, sliding-window causal attention) on 8 trn2 cores.

Sharding: 4096 tokens (B=2 x S=2048) split into 8 contiguous chunks of 512
(core = b*4 + chunk). Activations are kept d-major ([dim, token], dim on
partitions) so every matmul is weights-stationary with no transposes.
Attention needs a 256-token left halo of K/V per layer: layer 0 computes it
locally from the embedding gather; layers 1-3 AllGather the residual-stream
halo over 4-core groups, overlapped with the next layer's halo-independent
work (LN1/Q/K/V over own tokens). Sliding-window masking is additive (-3000)
and applied inside PSUM via an identity-matmul accumulate, so the scalar
engine's exp produces masked probabilities directly. Softmax division is
deferred: unnormalized attention output and per-(head,q) denominators are
collected, one batched reciprocal per q-block computes 1/den for all heads,
and an indicator-matrix matmul broadcasts it back over the d-major layout.
The final projection is token-sharded: each core computes the FULL vocab for
its own 512 tokens (no final AllGather); w_out streams from DRAM in 5-tile
groups, and the output bias is added host-side.
"""
import os
import sys

import numpy as np

for _p in ("/opt/trn_rl_repo", "/root/.axon_site/_ro/trn_rl_repo"):
    if os.path.isdir(_p) and _p not in sys.path:
        sys.path.insert(0, _p)

import concourse.bass as bass
import concourse.mybir as mybir
import concourse.tile as tile
from concourse import bacc
from concourse.bass import ts, ds
from concourse.bass_utils import run_bass_kernel_spmd
from concourse.masks import make_identity

F32 = mybir.dt.float32
F32R = mybir.dt.float32r
F16 = mybir.dt.float16
I32 = mybir.dt.int32
MDT = F16 if os.environ.get("KMMDT", "f16") == "f16" else F32R
AF = mybir.ActivationFunctionType
OP = mybir.AluOpType

B, S, V, D, H, NL, MLPD = 2, 2048, 32000, 512, 8, 4, 2048
DH = D // H            # 64
HALF = 256             # attention half-window (WIN // 2)
P = 128
NCORES = 8
CHUNK = 512            # own tokens per core
W = CHUNK + HALF       # 768 = halo + own
DT = D // P            # 4 d-tiles
MT = MLPD // P         # 16 mlp tiles
VN = V // P            # 250 vocab tiles (each core does full vocab x own tokens)
GL = 5                 # vocab tiles per DMA group
VG = VN // GL          # 50 groups (batched DMA: 5KB contiguous per partition)
NTOK = B * S           # 4096
GROUPS = [[0, 1, 2, 3], [4, 5, 6, 7]]
EXP_SHIFT = 2.0
MASK_BIAS = -3000.0    # additive mask; exp(SCALE*(s+MASK_BIAS)+EXP_SHIFT) == 0
SCALE = float(1.0 / np.sqrt(DH))

_CACHE = {}


# ================================================================ builder
def _build():
    nc = bacc.Bacc("TRN2", target_bir_lowering=False, debug=False,
                   num_devices=NCORES)

    ein = lambda n, sh, dt=F32: nc.dram_tensor(n, sh, dt, kind="ExternalInput")
    io = dict(
        wq=ein("wq", [NL, P, DT * D], MDT), wk=ein("wk", [NL, P, DT * D], MDT),
        wv=ein("wv", [NL, P, DT * D], MDT), wo=ein("wo", [NL, P, DT * D], MDT),
        w1=ein("w1", [NL, P, DT * MLPD], MDT), w2=ein("w2", [NL, P, MT * D], MDT),
        b1=ein("b1", [P, NL * MT]), b2=ein("b2", [P, NL * DT]),
        ln1_s=ein("ln1_s", [P, NL * DT]), ln1_b=ein("ln1_b", [P, NL * DT]),
        ln2_s=ein("ln2_s", [P, NL * DT]), ln2_b=ein("ln2_b", [P, NL * DT]),
        lnf_s=ein("lnf_s", [P, DT]), lnf_b=ein("lnf_b", [P, DT]),
        w_tiles=ein("w_tiles", [VG, P, GL * DT * P], MDT),
        embed=ein("embed", [V, D]),
        idx_in=ein("idx_in", [P, W // P], I32),
        pe_dm=ein("pe_dm", [P, DT * W]),
        masks=ein("masks", [P, 2 * 4 * 256], MDT),
        halo_offs=ein("halo_offs", [P, DT], I32),
        out=nc.dram_tensor("logits_vm", [VG, P, GL * CHUNK], F16, kind="ExternalOutput"),
    )
    if os.environ.get("KDEBUG") == "1":
        io["d_y"] = nc.dram_tensor("d_y", [D, CHUNK], F16, kind="ExternalOutput")
        io["d_yh"] = nc.dram_tensor("d_yh", [D, HALF], F16, kind="ExternalOutput")
        io["d_attr"] = nc.dram_tensor("d_attr", [D, CHUNK], F16, kind="ExternalOutput")
        io["d_rf"] = nc.dram_tensor("d_rf", [16, 256], F32, kind="ExternalOutput")
        io["d_x1"] = nc.dram_tensor("d_x1", [D, CHUNK], F32, kind="ExternalOutput")
        io["d_xh"] = nc.dram_tensor("d_xh", [D, HALF], F32, kind="ExternalOutput")
        io["d_xhp"] = nc.dram_tensor("d_xhp", [D, HALF], F32, kind="ExternalOutput")
        io["d_y2h"] = nc.dram_tensor("d_y2h", [D, HALF], F16, kind="ExternalOutput")

    with tile.TileContext(nc) as tc, nc.allow_low_precision(reason="f32r rounding"):
        _emit(nc, tc, io)
    nc.compile()
    return nc


def _emit(nc, tc, io):
    cpool = tc.alloc_tile_pool(name="const", bufs=1)
    xpool = tc.alloc_tile_pool(name="xres", bufs=1)
    ps_a = tc.alloc_tile_pool(name="ps_a", bufs=2, space="PSUM")
    ps_b = tc.alloc_tile_pool(name="ps_b", bufs=4, space="PSUM")
    ps_c = tc.alloc_tile_pool(name="ps_c", bufs=2, space="PSUM")
    drp = tc.alloc_tile_pool(name="drbounce", bufs=1, space="DRAM")

    # ------------------------------------------------ constants
    ones_f = cpool.tile([P, P], F32, tag="ones_f")
    nc.vector.memset(ones_f[:], 1.0)
    ones = cpool.tile([P, P], MDT, tag="ones")
    nc.vector.tensor_copy(out=ones[:], in_=ones_f[:])
    ones_r = cpool.tile([P, P], F32R, tag="ones_r")
    nc.vector.tensor_copy(out=ones_r[:], in_=ones_f[:])
    identm = cpool.tile([P, P], MDT, tag="identm")
    make_identity(nc, identm[:])
    negb = cpool.tile([P, 1], F32, tag="negb")
    nc.vector.memset(negb[:], EXP_SHIFT)
    epsb = cpool.tile([P, 1], F32, tag="epsb")
    nc.vector.memset(epsb[:], 1e-6)
    l1s = cpool.tile([P, NL, DT], F32, tag="l1s")
    l1b = cpool.tile([P, NL, DT], F32, tag="l1b")
    l2s = cpool.tile([P, NL, DT], F32, tag="l2s")
    l2b = cpool.tile([P, NL, DT], F32, tag="l2b")
    lfs = cpool.tile([P, DT], F32, tag="lfs")
    lfb = cpool.tile([P, DT], F32, tag="lfb")
    b1t = cpool.tile([P, NL, MT], F32, tag="b1t")
    b2t = cpool.tile([P, NL, DT], F32, tag="b2t")
    masks = cpool.tile([P, 2, 4, 256], MDT, tag="masks")
    hoffs = cpool.tile([P, DT], I32, tag="hoffs")

    def load_consts():
        nc.sync.dma_start(out=l1s[:], in_=io["ln1_s"].ap().rearrange("p (l t) -> p l t", l=NL))
        nc.sync.dma_start(out=l1b[:], in_=io["ln1_b"].ap().rearrange("p (l t) -> p l t", l=NL))
        nc.sync.dma_start(out=l2s[:], in_=io["ln2_s"].ap().rearrange("p (l t) -> p l t", l=NL))
        nc.sync.dma_start(out=l2b[:], in_=io["ln2_b"].ap().rearrange("p (l t) -> p l t", l=NL))
        nc.sync.dma_start(out=lfs[:], in_=io["lnf_s"].ap())
        nc.sync.dma_start(out=lfb[:], in_=io["lnf_b"].ap())
        nc.sync.dma_start(out=b1t[:], in_=io["b1"].ap().rearrange("p (l m) -> p l m", l=NL))
        nc.sync.dma_start(out=b2t[:], in_=io["b2"].ap().rearrange("p (l t) -> p l t", l=NL))
        nc.sync.dma_start(out=masks[:], in_=io["masks"].ap().rearrange("p (a b q) -> p a b q", a=2, b=4))
        nc.sync.dma_start(out=hoffs[:], in_=io["halo_offs"].ap())

    # residual stream (own 512 tokens, d-major, f32r so LN sum-matmuls can
    # consume it directly at full PE rate) + per-layer halo
    x = xpool.tile([P, DT, CHUNK], F32R, tag="x")
    xh = xpool.tile([P, DT, HALF], F32R, tag="xh")

    # ------------------------------------------------ embedding
    with tc.tile_pool(name="embed", bufs=1) as epool:
        ident = epool.tile([P, P], F32, tag="ident")
        make_identity(nc, ident[:])
        pe = epool.tile([P, DT, W], F32, tag="pe")
        nc.sync.dma_start(out=pe[:], in_=io["pe_dm"].ap().rearrange("p (t m) -> p t m", t=DT))
        idxt = epool.tile([P, W // P], I32, tag="idxt")
        nc.sync.dma_start(out=idxt[:], in_=io["idx_in"].ap())
        with tc.tile_pool(name="gath", bufs=2) as gpool:
            for g in range(W // P):
                gt = gpool.tile([P, D], F32, tag="gt")
                nc.gpsimd.indirect_dma_start(
                    out=gt[:], out_offset=None, in_=io["embed"].ap(),
                    in_offset=bass.IndirectOffsetOnAxis(ap=idxt[:, g:g + 1], axis=0),
                )
                for dt in range(DT):
                    pt = ps_a.tile([P, P], F32, tag="ps_a")
                    nc.tensor.transpose(pt[:], gt[:, ts(dt, P)], ident[:])
                    dst = xh[:, dt, ts(g, P)] if g < 2 else x[:, dt, ts(g - 2, P)]
                    nc.vector.tensor_add(out=dst, in0=pt[:], in1=pe[:, dt, ts(g, P)])

    load_consts()

    # ------------------------------------------------ layer pools
    lp = tc.alloc_tile_pool(name="layers", bufs=1)
    tp = tc.alloc_tile_pool(name="ltrans", bufs=2)
    lp3 = tc.alloc_tile_pool(name="ltrans3", bufs=3)
    vtp = tc.alloc_tile_pool(name="vtpool", bufs=1)

    # V tiles with a trailing ones column per head: PV matmul row DH
    # accumulates the softmax denominator for free. Ones written once.
    vt = [vtp.tile([P, H * (DH + 1)], MDT, tag=f"vt{t}", name=f"vt{t}")
          for t in range(W // P)]
    for t in range(W // P):
        vtv = vt[t][:].rearrange("p (h c) -> p h c", c=DH + 1)
        nc.vector.tensor_copy(out=vtv[:, :, DH:DH + 1], in_=ones[:, 0:H])

    def emit_ln(blocks):
        """LN over d (partition axis x DT), stage-pipelined across blocks.
        blocks: list of (src_fn(dt), dst_fn(dt), width, s_of, b_of, xdt)."""
        st = []
        for bi, (fn, dst, width, sof, bof, xd) in enumerate(blocks):
            spool = ps_a if bi % 2 == 0 else ps_b
            ptag = "ps_a" if bi % 2 == 0 else "ps_b"
            ones_x = ones_r if xd == F32R else ones
            sx = spool.tile([1, 512], F32, tag=ptag)
            sxx = spool.tile([1, 512], F32, tag=ptag)
            for dt in range(DT):
                xsq = lp3.tile([P, 512], MDT, tag="ln_xsq", bufs=2)
                nc.vector.tensor_mul(out=xsq[:, :width], in0=fn(dt), in1=fn(dt))
                nc.tensor.matmul(out=sx[:, :width], lhsT=ones_x[:, 0:1], rhs=fn(dt),
                                 start=(dt == 0), stop=(dt == DT - 1))
                nc.tensor.matmul(out=sxx[:, :width], lhsT=ones[:, 0:1], rhs=xsq[:, :width],
                                 start=(dt == 0), stop=(dt == DT - 1))
            st.append([sx, sxx])
        for bi, (fn, dst, width, sof, bof, xd) in enumerate(blocks):
            sx, sxx = st[bi]
            mu = lp3.tile([1, 512], MDT, tag="ln_mu", bufs=2)
            nc.vector.tensor_scalar_mul(out=mu[:, :width], in0=sx[:, :width], scalar1=1.0 / D)
            mu2 = lp3.tile([1, 512], F32, tag="ln_mu2", bufs=2)
            nc.vector.tensor_mul(out=mu2[:, :width], in0=mu[:, :width], in1=mu[:, :width])
            var = lp3.tile([1, 512], F32, tag="ln_var", bufs=2)
            # var = sxx/D - mu^2
            nc.vector.scalar_tensor_tensor(
                out=var[:, :width], in0=sxx[:, :width], scalar=1.0 / D,
                in1=mu2[:, :width], op0=OP.mult, op1=OP.subtract)
            sd = lp3.tile([1, 512], F32, tag="ln_sd", bufs=2)
            nc.scalar.activation(sd[:, :width], var[:, :width], AF.Sqrt, bias=epsb[0:1, :], scale=1.0)
            rt = lp3.tile([1, 512], F32, tag="ln_rt", bufs=2)
            nc.vector.reciprocal_approx_fast(out=rt[:, :width], in_=sd[:, :width])
            rstd = lp3.tile([1, 512], MDT, tag="ln_rstd", bufs=2)
            nc.vector.tensor_copy(out=rstd[:, :width], in_=rt[:, :width])
            st[bi] += [mu, rstd]
        for bi, (fn, dst, width, sof, bof, xd) in enumerate(blocks):
            sx, sxx, mu, rstd = st[bi]
            bpool = ps_c if bi % 2 == 0 else ps_b
            ptag = "ps_c" if bi % 2 == 0 else "ps_b"
            pmu = bpool.tile([P, 512], F32, tag=ptag)
            nc.tensor.matmul(out=pmu[:, :width], lhsT=ones[0:1, :], rhs=mu[:, :width],
                             start=True, stop=True)
            prs = bpool.tile([P, 512], F32, tag=ptag)
            nc.tensor.matmul(out=prs[:, :width], lhsT=ones[0:1, :], rhs=rstd[:, :width],
                             start=True, stop=True)
            st[bi] += [pmu, prs]
        for dt in range(DT):
            for bi, (fn, dst, width, sof, bof, xd) in enumerate(blocks):
                sx, sxx, mu, rstd, pmu, prs = st[bi]
                scr = lp3.tile([P, 512], F32, tag="ln_scr", bufs=2)
                nc.vector.tensor_sub(out=scr[:, :width], in0=fn(dt), in1=pmu[:, :width])
                nc.vector.tensor_mul(out=scr[:, :width], in0=scr[:, :width], in1=prs[:, :width])
                nc.vector.tensor_scalar(out=dst(dt), in0=scr[:, :width],
                                        scalar1=sof(dt), scalar2=bof(dt),
                                        op0=OP.mult, op1=OP.add)

    def load_w(dram_ap, tag_r, shape3, rpool=None):
        wr = (rpool or tp).tile(shape3, MDT, tag=tag_r)
        nc.sync.dma_start(out=wr[:], in_=dram_ap)
        return wr

    # ------------------------------------------------ transformer layers
    _knl = int(os.environ.get("KNL", NL))
    xh_pre = xpool.tile([P, DT, HALF], F16, tag="xh_pre")
    wmlp = {}
    for l in range(_knl):
        li = l % NL
        lpv = (l - 1) % NL
        s1 = lambda dt: l1s[:, li, dt:dt + 1]
        b1_ = lambda dt: l1b[:, li, dt:dt + 1]
        y = lp.tile([P, DT, CHUNK], MDT, tag="y")
        yh = lp.tile([P, DT, HALF], MDT, tag="yh")
        krh = lp.tile([P, DT, HALF], MDT, tag="krh")
        # LN1 over own tokens first: halo-independent; two blocks pipeline
        # the stat chains.
        emit_ln([(lambda dt: x[:, dt, 0:256], lambda dt: y[:, dt, 0:256], 256, s1, b1_, F32R),
                 (lambda dt: x[:, dt, 256:512], lambda dt: y[:, dt, 256:512], 256, s1, b1_, F32R)])

        # --- projections (weights stationary, d-major out), own tokens
        wq_r = load_w(io["wq"].ap()[li].rearrange("p (t m) -> p t m", t=DT), "wr", [P, DT, D])
        qr = lp.tile([P, DT, CHUNK], MDT, tag="qr")
        for do in range(DT):
            pq = ps_a.tile([P, CHUNK], F32, tag="ps_a")
            for dt in range(DT):
                nc.tensor.matmul(out=pq[:], lhsT=wq_r[:, dt, ts(do, P)],
                                 rhs=y[:, dt, :], start=(dt == 0), stop=(dt == DT - 1))
            nc.vector.tensor_copy(out=qr[:, do, :], in_=pq[:])

        wk_r = load_w(io["wk"].ap()[li].rearrange("p (t m) -> p t m", t=DT), "wr", [P, DT, D])
        kro = lp.tile([P, DT, CHUNK], MDT, tag="kro")
        for do in range(DT):
            pk = ps_a.tile([P, CHUNK], F32, tag="ps_a")
            for dt in range(DT):
                nc.tensor.matmul(out=pk[:], lhsT=wk_r[:, dt, ts(do, P)],
                                 rhs=y[:, dt, :], start=(dt == 0), stop=(dt == DT - 1))
            nc.vector.tensor_copy(out=kro[:, do, :], in_=pk[:])

        wv_r = load_w(io["wv"].ap()[li].rearrange("p (t m) -> p t m", t=DT), "wr", [P, DT, D])
        for t in range(2, W // P):
            pv = ps_a.tile([P, D], F32, tag="ps_a")
            for dt in range(DT):
                nc.tensor.matmul(out=pv[:], lhsT=y[:, dt, ts(t - 2, P)], rhs=wv_r[:, dt, :],
                                 start=(dt == 0), stop=(dt == DT - 1))
            vtv = vt[t][:].rearrange("p (h c) -> p h c", c=DH + 1)
            nc.vector.tensor_copy(out=vtv[:, :, 0:DH],
                                  in_=pv[:].rearrange("p (h c) -> p h c", c=DH))

        # --- sliding-window attention, deferred softmax division
        attru = lp.tile([P, DT, CHUNK], F32, tag="attru")
        attr = lp.tile([P, DT, CHUNK], MDT, tag="attr")

        def kslice(kt, r0, dto):
            if kt < 2:
                return krh[ds(r0, DH), dto, ts(kt, P)]
            return kro[ds(r0, DH), dto, ts(kt - 2, P)]

        def attn_qblock(qB):
            dga = lp3.tile([1, H * 256], F32, tag="dga", bufs=2)
            for h in range(H):
                r0 = (h % 2) * DH
                dto = h // 2
                pa = ps_c.tile([DH + 1, 256], F32, tag="ps_c")
                for jp in range(2):
                    pscore = ps_b.tile([P, 512], F32, tag="ps_b")
                    for jj in range(2):
                        j = jp * 2 + jj
                        kt = qB * 2 + j
                        nc.tensor.matmul(out=pscore[:, ts(jj, 256)],
                                         lhsT=kslice(kt, r0, dto),
                                         rhs=qr[ds(r0, DH), dto, ds(qB * 256, 256)],
                                         start=True, stop=False)
                        nc.tensor.matmul(out=pscore[:, ts(jj, 256)],
                                         lhsT=identm[:], rhs=masks[:, qB, j, :],
                                         start=False, stop=True)
                    ej = lp3.tile([P, 512], MDT, tag="ej", bufs=3)
                    nc.scalar.activation(ej[:], pscore[:], AF.Exp, bias=negb[:], scale=SCALE)
                    for jj in range(2):
                        j = jp * 2 + jj
                        kt = qB * 2 + j
                        nc.tensor.matmul(out=pa[:], lhsT=vt[kt][:, ds(h * (DH + 1), DH + 1)],
                                         rhs=ej[:, ts(jj, 256)], start=(j == 0), stop=(j == 3))
                nc.vector.tensor_copy(out=attru[ds(r0, DH), dto, ds(qB * 256, 256)],
                                      in_=pa[0:DH, :])
                nc.vector.tensor_copy(out=dga[:, ds(h * 256, 256)], in_=pa[DH:DH + 1, :])
            rf = lp3.tile([1, H * 256], F32, tag="rf", bufs=2)
            nc.vector.reciprocal_approx_fast(out=rf[:], in_=dga[:])
            rfh = lp3.tile([1, H * 256], MDT, tag="rfh", bufs=2)
            nc.vector.tensor_copy(out=rfh[:], in_=rf[:])
            for h in range(H):
                r0 = (h % 2) * DH
                dto = h // 2
                psc = ps_a.tile([DH, 256], F32, tag="ps_a")
                nc.tensor.matmul(out=psc[:], lhsT=ones[0:1, 0:DH],
                                 rhs=rfh[:, ds(h * 256, 256)],
                                 start=True, stop=True)
                nc.vector.tensor_mul(out=attr[ds(r0, DH), dto, ds(qB * 256, 256)],
                                     in0=attru[ds(r0, DH), dto, ds(qB * 256, 256)],
                                     in1=psc[:])

        # q-block 1 attends only to own keys: runs while the halo is in flight
        attn_qblock(1)

        # --- LN1 over halo + K/V halo (halo arrives via AllGather; consumed
        # as late as possible so the collective hides under own-token work)
        hx, hdt = (xh, F32R) if l == 0 else (xh_pre, F16)
        emit_ln([(lambda dt: hx[:, dt, :], lambda dt: yh[:, dt, :], HALF, s1, b1_, hdt)])
        for do in range(DT):
            pk = ps_a.tile([P, CHUNK], F32, tag="ps_a")
            for dt in range(DT):
                nc.tensor.matmul(out=pk[:, :HALF], lhsT=wk_r[:, dt, ts(do, P)],
                                 rhs=yh[:, dt, :], start=(dt == 0), stop=(dt == DT - 1))
            nc.vector.tensor_copy(out=krh[:, do, :], in_=pk[:, :HALF])
        for t in range(2):
            pv = ps_a.tile([P, D], F32, tag="ps_a")
            for dt in range(DT):
                nc.tensor.matmul(out=pv[:], lhsT=yh[:, dt, ts(t, P)], rhs=wv_r[:, dt, :],
                                 start=(dt == 0), stop=(dt == DT - 1))
            vtv = vt[t][:].rearrange("p (h c) -> p h c", c=DH + 1)
            nc.vector.tensor_copy(out=vtv[:, :, 0:DH],
                                  in_=pv[:].rearrange("p (h c) -> p h c", c=DH))

        attn_qblock(0)

        if l == 0 and "d_y" in io:
            nc.sync.dma_start(out=io["d_y"].ap().rearrange("(t p) m -> p t m", p=P), in_=y[:])
            nc.sync.dma_start(out=io["d_yh"].ap().rearrange("(t p) m -> p t m", p=P), in_=yh[:])
            nc.sync.dma_start(out=io["d_attr"].ap().rearrange("(t p) m -> p t m", p=P), in_=attr[:])

        # --- output projection + residual
        wo_r = load_w(io["wo"].ap()[li].rearrange("p (t m) -> p t m", t=DT), "wr", [P, DT, D])
        for do in range(DT):
            po = ps_a.tile([P, CHUNK], F32, tag="ps_a")
            for dt in range(DT):
                nc.tensor.matmul(out=po[:], lhsT=wo_r[:, dt, ts(do, P)],
                                 rhs=attr[:, dt, :], start=(dt == 0), stop=(dt == DT - 1))
            nc.vector.tensor_add(out=x[:, do, :], in0=x[:, do, :], in1=po[:])

        # --- LN2 + MLP
        y2 = lp.tile([P, DT, CHUNK], MDT, tag="y2")
        s2 = lambda dt: l2s[:, li, dt:dt + 1]
        b2_ = lambda dt: l2b[:, li, dt:dt + 1]
        emit_ln([(lambda dt: x[:, dt, 0:256], lambda dt: y2[:, dt, 0:256], 256, s2, b2_, F32R),
                 (lambda dt: x[:, dt, 256:512], lambda dt: y2[:, dt, 256:512], 256, s2, b2_, F32R)])

        pb = [ps_b.tile([P, CHUNK], F32, tag="ps_b", name=f"pb{i}") for i in range(DT)]
        w1r = lp.tile([P, DT, MLPD], MDT, tag="w1r")
        nc.sync.dma_start(out=w1r[:], in_=io["w1"].ap()[li].rearrange("p (t m) -> p t m", t=DT))
        w2r = lp.tile([P, MT, D], MDT, tag="w2r")
        nc.sync.dma_start(out=w2r[:], in_=io["w2"].ap()[li].rearrange("p (t m) -> p t m", t=MT))

        def emit_mlp2(m, hm):
            for do in range(DT):
                nc.tensor.matmul(out=pb[do][:], lhsT=w2r[:, m, ts(do, P)],
                                 rhs=hm[:], start=(m == 0), stop=(m == MT - 1))

        hist = []
        for m in range(MT):
            p1 = ps_a.tile([P, CHUNK], F32, tag="ps_a")
            for dt in range(DT):
                nc.tensor.matmul(out=p1[:], lhsT=w1r[:, dt, ts(m, P)],
                                 rhs=y2[:, dt, :], start=(dt == 0), stop=(dt == DT - 1))
            hm = lp3.tile([P, CHUNK], MDT, tag="hm", bufs=3)
            nc.scalar.activation(hm[:], p1[:], AF.Gelu_apprx_tanh,
                                 bias=b1t[:, li, m:m + 1], scale=1.0)
            hist.append((m, hm))
            if len(hist) > 2:
                emit_mlp2(*hist.pop(0))
        for mm_, hh_ in hist:
            emit_mlp2(mm_, hh_)
        # residual (+b2)
        for do in range(DT):
            nc.vector.scalar_tensor_tensor(
                out=x[:, do, :], in0=pb[do][:],
                scalar=b2t[:, li, do:do + 1], in1=x[:, do, :],
                op0=OP.add, op1=OP.add)
        if l == 0 and "d_x1" in io:
            xd = lp3.tile([P, DT, CHUNK], F32, tag="xdump", bufs=1)
            nc.vector.tensor_copy(out=xd[:], in_=x[:])
            nc.sync.dma_start(out=io["d_x1"].ap().rearrange("(t p) m -> p t m", p=P), in_=xd[:])
        if l < NL - 1:
            xhs = lp.tile([P, DT, HALF], F16, tag="xhs")
            nc.vector.tensor_copy(out=xhs[:], in_=x[:, :, ds(HALF, HALF)])
            agin = drp.tile([D, HALF], F16, tag=f"agin{l}")
            agout = drp.tile([len(GROUPS[0]) * D, HALF], F16, tag=f"agout{l}")
            nc.sync.dma_start(out=agin[:].rearrange("(t p) m -> p t m", p=P),
                              in_=xhs[:])
            nc.gpsimd.collective_compute(
                "AllGather", OP.bypass, replica_groups=GROUPS,
                ins=[agin.opt()], outs=[agout.opt()])
            for dt in range(DT):
                nc.gpsimd.indirect_dma_start(
                    out=xh_pre[:, dt, :], out_offset=None, in_=agout[:],
                    in_offset=bass.IndirectOffsetOnAxis(ap=hoffs[:, dt:dt + 1], axis=0))

    # ------------------------------------------------ final LN + logits
    # Each core computes the FULL vocab for its own 512 tokens: no final
    # AllGather; w_out streams tile-by-tile from DRAM, prefetched by the
    # pool double-buffering. Output bias is added host-side.
    yf = lp.tile([P, DT, CHUNK], MDT, tag="y")
    fs_ = lambda dt: lfs[:, dt:dt + 1]
    fb_ = lambda dt: lfb[:, dt:dt + 1]
    emit_ln([(lambda dt: x[:, dt, 0:256], lambda dt: yf[:, dt, 0:256], 256, fs_, fb_, F32R),
             (lambda dt: x[:, dt, 256:512], lambda dt: yf[:, dt, 256:512], 256, fs_, fb_, F32R)])

    vtp.release()
    lp3.release()
    tp.release()

    ps_c.release()
    ps_b.release()
    fps = tc.alloc_tile_pool(name="fps", bufs=4, space="PSUM")
    with tc.tile_pool(name="ftrans", bufs=3) as ftp, \
         tc.tile_pool(name="fout", bufs=3) as fop:
        for g in range(VG):
            fwr = ftp.tile([P, GL, DT, P], MDT, tag="fwr")
            nc.sync.dma_start(out=fwr[:],
                              in_=io["w_tiles"].ap()[g]
                              .rearrange("p (j t q) -> p j t q", j=GL, t=DT))
            ot = fop.tile([P, GL, CHUNK], F16, tag="fot")
            for j in range(GL):
                pf = fps.tile([P, CHUNK], F32, tag="fps")
                for dt in range(DT):
                    nc.tensor.matmul(out=pf[:], lhsT=fwr[:, j, dt, :], rhs=yf[:, dt, :],
                                     start=(dt == 0), stop=(dt == DT - 1))
                if (g * GL + j) % 2 == 0:
                    nc.vector.tensor_copy(out=ot[:, j, :], in_=pf[:])
                else:
                    nc.scalar.activation(ot[:, j, :], pf[:], AF.Copy)
            nc.sync.dma_start(out=io["out"].ap()[g],
                              in_=ot[:].rearrange("p j m -> p (j m)"))

    fps.release()
    lp.release()
    drp.release()
    ps_a.release()
    xpool.release()
    cpool.release()


# ================================================================ host side
def _pe_table():
    pos = np.arange(S, dtype=np.float32)[:, None]
    div = np.exp(np.arange(0, D, 2, dtype=np.float32) * -(np.log(10000.0) / D))
    pe = np.zeros((S, D), dtype=np.float32)
    pe[:, 0::2] = np.sin(pos * div)
    pe[:, 1::2] = np.cos(pos * div)
    return pe


def _in_maps(inputs):
    inp = np.asarray(inputs["inputs"]).astype(np.int32)
    ids = np.pad(inp, ((0, 0), (1, 0)))[:, :-1].astype(np.int32)
    pe = _pe_table()
    wout = np.asarray(inputs["w_out"], dtype=np.float32).astype(np.float16)
    def dmaj(a):
        # [X, (DT_, P)-rows, M] -> [X, P, DT_*M] device layout (d-major tiles)
        a = np.asarray(a)
        nl, dd, m = a.shape
        return np.ascontiguousarray(
            a.reshape(nl, dd // P, P, m).transpose(0, 2, 1, 3).reshape(nl, P, (dd // P) * m))

    def prow(a, tiles):
        # [.., tiles*P] -> [P, .. * tiles] per-partition rows
        a = np.asarray(a, np.float32).reshape(-1, tiles, P)
        return np.ascontiguousarray(a.transpose(2, 0, 1).reshape(P, -1))

    shared = {"embed": np.ascontiguousarray(np.asarray(inputs["embed"], np.float32))}
    for k in ("ln1_s", "ln1_b", "ln2_s", "ln2_b"):
        shared[k] = prow(inputs[k], DT)
    shared["b1"] = prow(inputs["b1"], MT)
    shared["b2"] = prow(inputs["b2"], DT)
    for k in ("wq", "wk", "wv", "wo", "w1"):
        shared[k] = dmaj(np.asarray(inputs[k], np.float32).astype(np.float16))
    shared["w2"] = dmaj(np.asarray(inputs["w2"], np.float32).astype(np.float16))
    shared["lnf_s"] = prow(np.asarray(inputs["lnf_s"], np.float32).reshape(1, D), DT)
    shared["lnf_b"] = prow(np.asarray(inputs["lnf_b"], np.float32).reshape(1, D), DT)
    # w_tiles[g, p, ((j*DT+dt)*P)+q] = w_out[dt*128+p, (g*GL+j)*128+q]
    shared["w_tiles"] = np.ascontiguousarray(
        wout.reshape(DT, P, VG, GL, P).transpose(2, 1, 3, 0, 4)
        .reshape(VG, P, GL * DT * P))

    maps = []
    qi = np.arange(256)[None, :]
    ki = np.arange(P)[:, None]
    for c in range(NCORES):
        b, ch = divmod(c, NCORES // B)
        t0 = ch * CHUNK
        lo = t0 - HALF
        ids768 = np.zeros(W, np.int32)
        pe768 = np.zeros((W, D), np.float32)
        s0 = max(0, lo)
        ids768[s0 - lo:] = ids[b, s0:t0 + CHUNK]
        pe768[s0 - lo:] = pe[s0:t0 + CHUNK]
        m = np.zeros((2, 4, P, 256), np.float16)
        for qB in range(2):
            for j in range(4):
                w = 256 + qi - (j * P + ki)      # u_q - u_k
                ok = (w >= 0) & (w <= HALF)
                if ch == 0:
                    ok = ok & ((lo + qB * 256 + j * P + ki) >= 0)
                m[qB, j] = np.where(ok, 0.0, MASK_BIAS).astype(np.float16)
        src = ch - 1 if ch > 0 else 0
        hoffs = (src * D + np.arange(DT)[None, :] * P
                 + np.arange(P)[:, None]).astype(np.int32)
        mp = dict(shared)
        mp.update(
            idx_in=np.ascontiguousarray(ids768.reshape(W // P, P).T),
            pe_dm=np.ascontiguousarray(
                pe768.T.reshape(DT, P, W).transpose(1, 0, 2).reshape(P, DT * W)),
            masks=np.ascontiguousarray(
                m.transpose(2, 0, 1, 3).reshape(P, 2 * 4 * 256)),
            halo_offs=hoffs)
        maps.append(mp)
    return maps


def kernel(**inputs):
    nc = _CACHE.get("nc")
    if nc is None:
        nc = _build()
        _CACHE["nc"] = nc
    maps = _in_maps(inputs)
    res = run_bass_kernel_spmd(nc, maps, list(range(NCORES))).results
    bout = np.asarray(inputs["b_out"], dtype=np.float32)
    full = np.empty((NTOK, V), np.float32)
    for c in range(NCORES):
        lv = (res[c]["logits_vm"].reshape(VG, P, GL, CHUNK)
              .transpose(0, 2, 1, 3).reshape(V, CHUNK))
        full[c * CHUNK:(c + 1) * CHUNK, :] = lv.T.astype(np.float32) + bout[None, :]
    return full.reshape(B, S, V)
